# revision 1
# baseline (speedup 1.0000x reference)
"""GCN edge-aggregation kernel for 8 Trainium2 NeuronCores.

Math (see nn_GCNEdge): h = relu((segment_sum(edge_data, dst) / max(count,1)) @ W.T + b)

Strategy
--------
Host-side (sharding/layout only — all arithmetic happens on device):
  * Nodes are split contiguously across the 8 cores (12544 = 98 blocks of 128
    nodes per core; 8*12544 = 100352 >= 100000).
  * Each edge is routed to the core/block owning its destination node (CSR-style
    destination binning).  Within a block, edges occupy sequential slots; each
    block is padded to K_CHUNKS*128 slots so the device program is data-independent.
  * Edge features are shipped as a bf16 hi/lo pair (hi = bf16(x),
    lo = bf16(x - hi)) so the on-device f32-accumulated matmuls reconstruct
    ~fp32 precision while streaming at bf16 rates.  A constant-1 column rides
    along for the degree counts.

Device-side (per core, per 128-node block):
  * One-hot matrix of local node ids (DVE is_equal against an iota row),
  * PE matmul-accumulate onehot.T @ [x_hi | 1 | x_lo | 0] into PSUM -> per-node
    feature sums (hi+lo parts) and counts,
  * mean = sums * reciprocal(max(count, 1)),
  * PE transpose, then out = relu(W @ agg.T + b) via a second matmul with the
    (pre-transposed) weight as the stationary operand; output stays transposed
    [out_feat, node] and is un-transposed on the host.

No collectives are needed: output shards are disjoint.
"""

import numpy as np
import ml_dtypes

BF16 = ml_dtypes.bfloat16

N_NODES = 100000
N_EDGES = 1600000
F = 128
N_CORES = 8
BLK = 128                       # nodes per block
BLOCKS_PER_CORE = 98
TOTAL_BLOCKS = N_CORES * BLOCKS_PER_CORE        # 784
NODES_PER_CORE = BLOCKS_PER_CORE * BLK          # 12544
K_CHUNKS = 18                   # 128-edge chunks per block (capacity 2304 edges)

_module_cache = {}


def _build_module(K):
    import concourse.mybir as mybir
    import concourse.tile as tile
    from concourse import bacc

    f32 = mybir.dt.float32
    bf16 = mybir.dt.bfloat16
    RB = K * 128                 # edge slots per block
    SLOTS = BLOCKS_PER_CORE * RB

    nc = bacc.Bacc("TRN2", target_bir_lowering=False, debug=False)
    # xe rows are (block, partition); each row is that partition's K chunks of
    # 258 bf16 values laid contiguously -> 9KB-contiguous DMA descriptors.
    xe = nc.dram_tensor("xe", [BLOCKS_PER_CORE * 128, K * 258], bf16, kind="ExternalInput")
    lid = nc.dram_tensor("lid", [128, BLOCKS_PER_CORE * K], bf16, kind="ExternalInput")
    wt = nc.dram_tensor("wt", [128, 128], f32, kind="ExternalInput")
    bias = nc.dram_tensor("bias", [128, 1], f32, kind="ExternalInput")
    ident = nc.dram_tensor("ident", [128, 128], f32, kind="ExternalInput")
    # iota value pattern tiled K times: iotar[p, c*128 + f] = f
    iotar = nc.dram_tensor("iotar", [128, K * 128], bf16, kind="ExternalInput")
    out = nc.dram_tensor("out", [128, BLOCKS_PER_CORE * 128], f32, kind="ExternalOutput")

    xe_ap = xe.ap()
    out_ap = out.ap()

    with tile.TileContext(nc) as tc:
        with (
            tc.tile_pool(name="const", bufs=1) as cpool,
            tc.tile_pool(name="xp", bufs=6) as xpool,
            tc.tile_pool(name="ohp", bufs=8) as ohpool,
            tc.tile_pool(name="ep", bufs=3) as epool,
            tc.tile_pool(name="psS", bufs=4, space="PSUM") as psS,
            tc.tile_pool(name="psT", bufs=2, space="PSUM") as psT,
            tc.tile_pool(name="psO", bufs=2, space="PSUM") as psO,
        ):
            wt_t = cpool.tile([128, 128], f32)
            nc.sync.dma_start(wt_t[:], wt.ap()[:])
            bias_t = cpool.tile([128, 1], f32)
            nc.sync.dma_start(bias_t[:], bias.ap()[:])
            id_t = cpool.tile([128, 128], f32)
            nc.sync.dma_start(id_t[:], ident.ap()[:])
            iotar_t = cpool.tile([128, K * 128], bf16)
            nc.sync.dma_start(iotar_t[:], iotar.ap()[:])
            lid_t = cpool.tile([128, BLOCKS_PER_CORE * K], bf16)
            nc.sync.dma_start(lid_t[:], lid.ap()[:])

            group_pT = {}

            def emit_matmuls(b, xt, oh):
                ps = psS.tile([128, 258], f32, name=f"ps{b}", tag="ps")
                for c in range(K):
                    nc.tensor.matmul(
                        ps[:],
                        lhsT=oh[:, c * 128:(c + 1) * 128],
                        rhs=xt[:, c * 258:(c + 1) * 258],
                        start=(c == 0),
                        stop=(c == K - 1),
                    )
                return ps

            def emit_pscopy(b, ps):
                # Drain PSUM to SBUF with a single ACT copy (emitted one block
                # after the accumulation finished, so the ACT queue never
                # blocks on it) — frees the PSUM bank early; the lagged
                # epilogue then reads SBUF only.
                s_sb = epool.tile([128, 257], f32, name=f"s{b}", tag="s_sb", bufs=5)
                nc.scalar.copy(s_sb[:], ps[:, 0:257])
                return s_sb

            def emit_epilogue(b, ps):
                # counts live in ps[:,128] (the lo-side count column is all
                # zeros by construction), so no hi+lo add is needed for them.
                # No max(count,1) guard: the host guarantees every real node
                # has count > 0 (injecting 1e-30-weight phantom edges if
                # needed); padding nodes divide by zero -> NaN columns that
                # the host slices off.  Keeping DVE's per-block work to this
                # single tiny op is what lets the wide one-hot builds stream.
                rec = epool.tile([128, 1], f32, name=f"rec{b}", tag="rec")
                nc.vector.reciprocal(rec[:], ps[:, 128:129])
                # agg = (S_hi + S_lo)/count: t1 = S_hi*rec on ACT, then one
                # fused DVE op: agg = S_lo*rec + t1.
                t1 = epool.tile([128, 128], f32, name=f"t1{b}", tag="t1")
                nc.scalar.activation(
                    t1[:], ps[:, 0:128],
                    mybir.ActivationFunctionType.Copy, scale=rec[:, 0:1],
                )
                agg = epool.tile([128, 128], f32, name=f"agg{b}", tag="agg")
                nc.vector.scalar_tensor_tensor(
                    out=agg[:],
                    in0=ps[:, 129:257],
                    scalar=rec[:, 0:1],
                    in1=t1[:],
                    op0=mybir.AluOpType.mult,
                    op1=mybir.AluOpType.add,
                )
                # NOTE: `ps` here is the SBUF-staged copy (s_sb), not PSUM.
                j = b % 4
                if j == 0:
                    group_pT["t"] = psT.tile([128, 512], f32, name=f"pT{b}", tag="pT")
                pT = group_pT["t"]
                nc.tensor.transpose(pT[:, j * 128:(j + 1) * 128], agg[:], id_t[:])
                if j == 3 or b == BLOCKS_PER_CORE - 1:
                    g0 = (b // 4) * 4
                    gw = (b + 1 - g0) * 128
                    aggT = epool.tile([128, 512], f32, name=f"aggT{b}", tag="aggT", bufs=2)
                    nc.scalar.copy(aggT[:, 0:gw], pT[:, 0:gw])
                    pO = psO.tile([128, 512], f32, name=f"pO{b}", tag="pO")
                    nc.tensor.matmul(
                        pO[:, 0:gw], lhsT=wt_t[:], rhs=aggT[:, 0:gw],
                        start=True, stop=True,
                    )
                    ot = epool.tile([128, 512], f32, name=f"ot{b}", tag="ot", bufs=2)
                    nc.scalar.activation(
                        ot[:, 0:gw], pO[:, 0:gw],
                        mybir.ActivationFunctionType.Relu,
                        bias=bias_t[:, 0:1], scale=1.0,
                    )
                    nc.sync.dma_start(out_ap[:, g0 * 128:(b + 1) * 128], ot[:, 0:gw])

            # Software-pipelined emission. Every engine queue is strict
            # in-order, so an op gated on *fresh* upstream state stalls the
            # whole queue behind it. Stagger each stage so, by the time a
            # queue reaches an op, its dependencies are blocks old:
            #   iter b:  DMA xt(b) | one-hot TT(b) | PE matmuls(b-1)
            #            | PSUM->SBUF drain of (b-2) | epilogue of (b-5)
            pending = {}
            pending_ps = {}
            pending_s = {}
            for b in range(BLOCKS_PER_CORE):
                xt = xpool.tile([128, K * 258], bf16, name=f"xt{b}", tag="xt")
                nc.sync.dma_start(xt[:], xe_ap[b * 128:(b + 1) * 128, :])
                oh = ohpool.tile([128, K * 128], bf16, name=f"oh{b}", tag="oh")
                nc.vector.tensor_tensor(
                    out=oh[:].rearrange("p (c f) -> p c f", c=K),
                    in0=iotar_t[:].rearrange("p (c f) -> p c f", c=K),
                    in1=lid_t[:, b * K:(b + 1) * K].to_broadcast([128, K, 128]),
                    op=mybir.AluOpType.is_equal,
                )
                pending[b] = (xt, oh)
                if b >= 1:
                    pending_ps[b - 1] = emit_matmuls(b - 1, *pending.pop(b - 1))
                if b >= 2:
                    pending_s[b - 2] = emit_pscopy(b - 2, pending_ps.pop(b - 2))
                if b >= 5:
                    emit_epilogue(b - 5, pending_s.pop(b - 5))
            last = BLOCKS_PER_CORE - 1
            pending_ps[last] = emit_matmuls(last, *pending.pop(last))
            for bb in sorted(pending_ps):
                pending_s[bb] = emit_pscopy(bb, pending_ps.pop(bb))
            for bb in sorted(pending_s):
                emit_epilogue(bb, pending_s.pop(bb))

    nc.compile()
    return nc


def _get_module(K):
    if K not in _module_cache:
        _module_cache[K] = _build_module(K)
    return _module_cache[K]


def prepare_inputs(edge_data, dst, W, b):
    """Host-side sharding: route each edge to the core/block owning dst."""
    edge_data = np.asarray(edge_data, dtype=np.float32)
    dst = np.asarray(dst)
    W = np.asarray(W, dtype=np.float32)
    b = np.asarray(b, dtype=np.float32)
    E = dst.shape[0]

    # The device kernel divides by the raw count (no max(count,1) guard).
    # Give any zero-degree real node a phantom edge with zero features and a
    # 1e-30 "count" weight: sums stay exactly 0, so mean = 0/1e-30 = 0, which
    # matches the reference's 0/max(0,1).
    node_cnt = np.bincount(dst, minlength=N_NODES)[:N_NODES]
    zeros = np.nonzero(node_cnt == 0)[0]
    n_real = E
    if len(zeros):
        dst = np.concatenate([dst, zeros.astype(dst.dtype)])
        E = dst.shape[0]

    blk = (dst.astype(np.int64)) >> 7                 # destination block id
    cnt = np.bincount(blk, minlength=TOTAL_BLOCKS)
    K = max(K_CHUNKS, int(np.ceil(cnt.max() / 128)))
    RB = K * 128
    TOT = TOTAL_BLOCKS * RB

    starts = np.zeros(TOTAL_BLOCKS, np.int64)
    np.cumsum(cnt[:-1], out=starts[1:])
    order = np.argsort(blk, kind="stable")
    rank = np.empty(E, np.int64)
    rank[order] = np.arange(E, dtype=np.int64) - np.repeat(starts, cnt)
    slot = blk * RB + rank

    X = np.zeros((TOT, 258), BF16)
    xh = edge_data.astype(BF16)
    X[slot[:n_real], 0:128] = xh
    X[slot[:n_real], 128] = BF16(1.0)
    X[slot[:n_real], 129:257] = (edge_data - xh.astype(np.float32)).astype(BF16)
    if len(zeros):
        X[slot[n_real:], 128] = BF16(1e-30)
    # [block, chunk, partition, feat] -> [block, partition, chunk*feat] so each
    # SBUF partition's data is one long contiguous HBM run (big DMA descriptors).
    X = np.ascontiguousarray(
        X.reshape(TOTAL_BLOCKS, K, 128, 258).transpose(0, 2, 1, 3)
    ).reshape(N_CORES, BLOCKS_PER_CORE * 128, K * 258)

    lid_f = np.full(TOT, -1.0, np.float32)
    lid_f[slot] = (dst & 127).astype(np.float32)
    lid_all = (
        lid_f.reshape(N_CORES, BLOCKS_PER_CORE, K, 128)
        .transpose(0, 3, 1, 2)
        .reshape(N_CORES, 128, BLOCKS_PER_CORE * K)
        .astype(BF16)
    )
    wt = np.ascontiguousarray(W.T)
    bias = np.ascontiguousarray(b.reshape(128, 1))
    ident = np.eye(128, dtype=np.float32)
    iotar = np.ascontiguousarray(
        np.broadcast_to(
            np.arange(128, dtype=np.float32), (128, K, 128)
        ).reshape(128, K * 128)
    ).astype(BF16)

    in_maps = [
        {
            "xe": np.ascontiguousarray(X[c]),
            "lid": np.ascontiguousarray(lid_all[c]),
            "wt": wt,
            "bias": bias,
            "ident": ident,
            "iotar": iotar,
        }
        for c in range(N_CORES)
    ]
    return K, in_maps


def run(edge_data, dst, W, b, trace=False, tmpdir=None):
    from concourse.bass_utils import run_bass_kernel_spmd

    K, in_maps = prepare_inputs(edge_data, dst, W, b)
    nc = _get_module(K)
    res = run_bass_kernel_spmd(
        nc, in_maps, core_ids=list(range(N_CORES)), trace=trace, tmpdir=tmpdir,
    )
    outs = [res.results[c]["out"].T for c in range(N_CORES)]   # [12544, 128] each
    full = np.concatenate(outs, axis=0)[:N_NODES]
    return np.ascontiguousarray(full, dtype=np.float32), res


def kernel(edge_data, dst, W, b):
    out, _ = run(edge_data, dst, W, b, trace=False)
    return out



# revision 3
# speedup vs baseline: 2.3044x; 2.3044x over previous
"""GCN edge-aggregation kernel for 8 Trainium2 NeuronCores.

Math (see nn_GCNEdge): h = relu((segment_sum(edge_data, dst) / max(count,1)) @ W.T + b)

Strategy (v2 — bf16 payload + 32-node groups + PE column tiling)
----------------------------------------------------------------
Host-side (sharding/layout only):
  * 100352 node slots = 8 cores x 98 blocks x 4 groups x 32 nodes.  Nodes are
    assigned to groups by degree-balanced LPT packing so that each group's
    edge count fits a *static* chunk budget of (4,4,4,5) chunks of 128 edges
    (tight groups <=512 edges, the loose group <=640).  Padding is ~6%
    (vs ~13% for destination-determined binning), and every block has exactly
    K=17 chunks -> one compiled program for all cores.
  * Edge features ship as single bf16 (error budget measured: full-chain
    rel err ~4.6e-3 vs the 2e-2 gate), halving HBM traffic vs an f32-accurate
    hi/lo pair.  Reciprocal mean weights (1/max(count,1)) are index-derived
    metadata and ship as a tiny [128 x 98] f32 tensor.
  * Edge features for 7 blocks are packed contiguously per SBUF partition so
    each input dma_start moves ~3.8 MB with 30 KB contiguous descriptors.

Device-side (per core, per 128-node block):
  * One-hot build on DVE is only 32 wide (is_equal against a tiled iota row):
    4x less DVE work than a 128-wide one-hot — this was the hidden second
    bottleneck of the previous version.
  * 17 matmuls accumulate oh_chunk.T @ x_chunk into a [128,128] f32 PSUM
    tile using PE 128x32 column tiling: each group owns PSUM partition
    quadrant 32g, so the 4 groups' weight loads overlap other groups'
    matmuls (round-robin emission).
  * ACT drains PSUM with the per-node reciprocal scale (mean), PE transposes
    4 blocks into a [128,512] tile, then out = relu(W @ agg.T + b) with a
    bf16 W; bf16 output columns staged [128,2048] and DMA'd per 16 blocks.

No collectives: output shards are disjoint; host inverts the node permutation.
"""

import numpy as np
import ml_dtypes

BF16 = ml_dtypes.bfloat16

N_NODES = 100000
N_EDGES = 1600000
F = 128
N_CORES = 8
BLOCKS = 98                        # blocks per core
GPB = 4                            # groups per block
GW = 32                            # nodes per group
CHUNKS = (4, 4, 4, 5)              # 128-edge chunks per group (static)
CHUNK_BASE = (0, 4, 8, 12)
K = 17                             # chunks per block
RB = K * 128                       # edge slots per block
MEGA = 7                           # blocks per input DMA
N_MEGA = BLOCKS // MEGA            # 14
NODES_PER_CORE = BLOCKS * 128      # 12544
NPAD = N_CORES * NODES_PER_CORE    # 100352
N_GROUPS = N_CORES * BLOCKS * GPB  # 3136
GROUP_CAPS = (512, 512, 512, 640)
GROUP_SLOT_BASE = (0, 512, 1024, 1536)

_module_cache = {}


def _build_module():
    import concourse.mybir as mybir
    import concourse.tile as tile
    from concourse import bacc

    f32 = mybir.dt.float32
    bf16 = mybir.dt.bfloat16

    nc = bacc.Bacc("TRN2", target_bir_lowering=False, debug=False)
    xe = nc.dram_tensor("xe", [N_MEGA * 128, MEGA * K * 128], bf16, kind="ExternalInput")
    lid = nc.dram_tensor("lid", [128, BLOCKS * K], bf16, kind="ExternalInput")
    recv = nc.dram_tensor("recv", [128, BLOCKS], f32, kind="ExternalInput")
    iotar = nc.dram_tensor("iotar", [128, K * GW], bf16, kind="ExternalInput")
    wt = nc.dram_tensor("wt", [128, 128], bf16, kind="ExternalInput")
    bias = nc.dram_tensor("bias", [128, 1], f32, kind="ExternalInput")
    ident = nc.dram_tensor("ident", [128, 128], f32, kind="ExternalInput")
    out = nc.dram_tensor("out", [128, BLOCKS * 128], bf16, kind="ExternalOutput")

    xe_ap = xe.ap()
    out_ap = out.ap()

    with tile.TileContext(nc) as tc:
        with (
            tc.tile_pool(name="const", bufs=1) as cpool,
            tc.tile_pool(name="xp", bufs=3) as xpool,
            tc.tile_pool(name="ohp", bufs=4) as ohpool,
            tc.tile_pool(name="ep", bufs=4) as epool,
            tc.tile_pool(name="stp", bufs=2) as stpool,
            tc.tile_pool(name="psS", bufs=4, space="PSUM") as psS,
            tc.tile_pool(name="psT", bufs=2, space="PSUM") as psT,
            tc.tile_pool(name="psO", bufs=2, space="PSUM") as psO,
        ):
            wt_t = cpool.tile([128, 128], bf16)
            nc.sync.dma_start(wt_t[:], wt.ap()[:])
            bias_t = cpool.tile([128, 1], f32)
            nc.sync.dma_start(bias_t[:], bias.ap()[:])
            id_t = cpool.tile([128, 128], f32)
            nc.sync.dma_start(id_t[:], ident.ap()[:])
            iotar_t = cpool.tile([128, K * GW], bf16)
            nc.sync.dma_start(iotar_t[:], iotar.ap()[:])
            lid_t = cpool.tile([128, BLOCKS * K], bf16)
            nc.sync.dma_start(lid_t[:], lid.ap()[:])
            rec_t = cpool.tile([128, BLOCKS], f32)
            nc.sync.dma_start(rec_t[:], recv.ap()[:])

            xt_tiles = {}

            def emit_dma(m):
                xt = xpool.tile([128, MEGA * K * 128], bf16, name=f"xt{m}", tag="xt")
                nc.sync.dma_start(xt[:], xe_ap[m * 128:(m + 1) * 128, :])
                xt_tiles[m] = xt

            def emit_onehot(b):
                oh = ohpool.tile([128, K * GW], bf16, name=f"oh{b}", tag="oh")
                nc.vector.tensor_tensor(
                    out=oh[:].rearrange("p (c f) -> p c f", c=K),
                    in0=iotar_t[:].rearrange("p (c f) -> p c f", c=K),
                    in1=lid_t[:, b * K:(b + 1) * K].to_broadcast([128, K, GW]),
                    op=mybir.AluOpType.is_equal,
                )
                return oh

            def emit_mms(b, oh):
                ps = psS.tile([128, 128], f32, name=f"ps{b}", tag="ps")
                xt = xt_tiles[b // MEGA]
                xoff = (b % MEGA) * K * 128
                # Round-robin across groups: consecutive matmuls target
                # different PE column tiles, so each weight load overlaps the
                # previous group's matmul.
                for r in range(max(CHUNKS)):
                    for g in range(GPB):
                        if r >= CHUNKS[g]:
                            continue
                        c = CHUNK_BASE[g] + r
                        nc.tensor.matmul(
                            ps[g * GW:(g + 1) * GW, :],
                            lhsT=oh[:, c * GW:(c + 1) * GW],
                            rhs=xt[:, xoff + c * 128: xoff + (c + 1) * 128],
                            start=(r == 0),
                            stop=(r == CHUNKS[g] - 1),
                            tile_position=(0, g * GW),
                        )
                return ps

            def emit_agg(b, ps):
                agg = epool.tile([128, 128], f32, name=f"agg{b}", tag="agg")
                nc.scalar.activation(
                    agg[:], ps[:],
                    mybir.ActivationFunctionType.Copy,
                    scale=rec_t[:, b:b + 1],
                )
                return agg

            stage = {}

            def emit_group(k, aggs):
                # blocks 4k .. 4k+nb-1 -> transpose, W matmul, relu, stage
                b0 = 4 * k
                nb = min(4, BLOCKS - b0)
                gw = nb * 128
                pT = psT.tile([128, 512], f32, name=f"pT{k}", tag="pT")
                for j in range(nb):
                    nc.tensor.transpose(
                        pT[:, j * 128:(j + 1) * 128], aggs[b0 + j][:], id_t[:]
                    )
                aggT = epool.tile([128, 512], bf16, name=f"aggT{k}", tag="aggT", bufs=2)
                nc.scalar.copy(aggT[:, 0:gw], pT[:, 0:gw])
                pO = psO.tile([128, 512], f32, name=f"pO{k}", tag="pO")
                nc.tensor.matmul(
                    pO[:, 0:gw], lhsT=wt_t[:], rhs=aggT[:, 0:gw],
                    start=True, stop=True,
                )
                s = k // 4
                if k % 4 == 0:
                    stage["t"] = stpool.tile([128, 2048], bf16, name=f"st{s}", tag="st")
                st = stage["t"]
                soff = (k % 4) * 512
                nc.scalar.activation(
                    st[:, soff:soff + gw], pO[:, 0:gw],
                    mybir.ActivationFunctionType.Relu,
                    bias=bias_t[:, 0:1], scale=1.0,
                )
                if k % 4 == 3 or b0 + nb == BLOCKS:
                    w = soff + gw
                    nc.sync.dma_start(out_ap[:, s * 2048: s * 2048 + w], st[:, 0:w])

            # Software-pipelined emission (strict in-order engine queues):
            #   iter b:  [mega DMA] | one-hot(b) | PE matmuls(b-1)
            #            | ACT mean-drain(b-2) | group stage per 4 blocks
            pend_oh = {}
            pend_ps = {}
            pend_agg = {}
            for b in range(BLOCKS):
                if b % MEGA == 0:
                    m = b // MEGA
                    if m == 0:
                        emit_dma(0)
                        emit_dma(1)
                    elif m + 1 < N_MEGA:
                        emit_dma(m + 1)
                pend_oh[b] = emit_onehot(b)
                if b >= 1:
                    pend_ps[b - 1] = emit_mms(b - 1, pend_oh.pop(b - 1))
                if b >= 2:
                    pend_agg[b - 2] = emit_agg(b - 2, pend_ps.pop(b - 2))
                if b >= 5 and (b - 5) % 4 == 0:
                    k = (b - 5) // 4
                    emit_group(k, pend_agg)
                    for bb in range(4 * k, 4 * k + 4):
                        pend_agg.pop(bb)
            last = BLOCKS - 1
            pend_ps[last] = emit_mms(last, pend_oh.pop(last))
            for bb in sorted(pend_ps):
                pend_agg[bb] = emit_agg(bb, pend_ps.pop(bb))
            emit_group(24, pend_agg)

    nc.compile()
    return nc


def _get_module():
    if "m" not in _module_cache:
        _module_cache["m"] = _build_module()
    return _module_cache["m"]


def _pack_nodes(deg):
    """Degree-balanced LPT assignment of NPAD node slots to 3136 groups.

    Returns assign_g[node] -> global group id  (gid = ((core*98+blk)*4+g)).
    Tight groups (g<3) must stay <=512 edges, loose (g==3) <=640; every
    group gets exactly 32 nodes.
    """
    import heapq

    order = np.argsort(-deg, kind="stable")
    bias = np.where(np.arange(N_GROUPS) % GPB == GPB - 1, -24.0, 0.0)
    load = np.zeros(N_GROUPS)
    cnt = np.zeros(N_GROUPS, np.int32)
    heap = [(bias[j], j) for j in range(N_GROUPS)]
    heapq.heapify(heap)
    assign_g = np.empty(NPAD, np.int32)
    for n in order:
        d = deg[n]
        while True:
            _, j = heapq.heappop(heap)
            if cnt[j] < GW:
                break
        assign_g[n] = j
        cnt[j] += 1
        load[j] += d
        if cnt[j] < GW:
            heapq.heappush(heap, (load[j] + bias[j], j))

    caps = np.asarray(GROUP_CAPS)[np.arange(N_GROUPS) % GPB]
    # Repair pass (defensive; LPT stays well under caps for this data):
    # swap small nodes out of overloaded groups into the lightest groups.
    for _ in range(64):
        over = np.nonzero(load > caps)[0]
        if not len(over):
            break
        for j in over:
            members = np.nonzero(assign_g == j)[0]
            excess = load[j] - caps[j]
            victims = members[np.argsort(deg[members])]
            tgt = np.argsort(load + bias)
            for v in victims:
                if excess <= 0:
                    break
                for j2 in tgt[:16]:
                    if j2 == j:
                        continue
                    m2 = np.nonzero(assign_g == j2)[0]
                    small = m2[np.argmin(deg[m2])]
                    if deg[small] < deg[v] and load[j2] + deg[v] - deg[small] <= caps[j2]:
                        assign_g[v], assign_g[small] = j2, j
                        delta = deg[v] - deg[small]
                        load[j] -= delta
                        load[j2] += delta
                        excess -= delta
                        break
    assert (load <= caps).all(), "group packing infeasible"
    return assign_g


def prepare_inputs(edge_data, dst, W, b):
    """Host-side sharding: degree-balanced routing of edges to core/block/group."""
    edge_data = np.asarray(edge_data, dtype=np.float32)
    dst = np.asarray(dst).astype(np.int64)
    W = np.asarray(W, dtype=np.float32)
    b = np.asarray(b, dtype=np.float32)
    E = dst.shape[0]

    deg = np.bincount(dst, minlength=NPAD).astype(np.int64)
    assign_g = _pack_nodes(deg)

    # local id (0..31) of each node within its group, slot->node map
    node_order = np.argsort(assign_g, kind="stable")
    lcl = np.empty(NPAD, np.int32)
    lcl[node_order] = np.arange(NPAD, dtype=np.int32) % GW
    nodemap = node_order  # nodemap[gid*32 + l] = node id

    # edge -> slot
    gid_e = assign_g[dst]
    cnt_g = np.bincount(gid_e, minlength=N_GROUPS)
    starts = np.zeros(N_GROUPS, np.int64)
    np.cumsum(cnt_g[:-1], out=starts[1:])
    eorder = np.argsort(gid_e, kind="stable")
    rank = np.empty(E, np.int64)
    rank[eorder] = np.arange(E, dtype=np.int64) - np.repeat(starts, cnt_g)
    blk_of_g = gid_e // GPB                       # core*98 + block
    g_of_g = gid_e % GPB
    slot = blk_of_g * RB + np.asarray(GROUP_SLOT_BASE)[g_of_g] + rank

    TOT = N_CORES * BLOCKS * RB
    X = np.zeros((TOT, 128), BF16)
    X[slot] = edge_data.astype(BF16)
    lid_f = np.full(TOT, -1.0, np.float32)
    lid_f[slot] = lcl[dst]

    # [core, block(=mega*7+j), chunk, part, feat] -> [core, mega, part, j, chunk, feat]
    X = np.ascontiguousarray(
        X.reshape(N_CORES, N_MEGA, MEGA, K, 128, 128).transpose(0, 1, 4, 2, 3, 5)
    ).reshape(N_CORES, N_MEGA * 128, MEGA * K * 128)
    lid_all = (
        lid_f.reshape(N_CORES, BLOCKS, K, 128)
        .transpose(0, 3, 1, 2)
        .reshape(N_CORES, 128, BLOCKS * K)
        .astype(BF16)
    )
    rec_all = (1.0 / np.maximum(deg, 1)).astype(np.float32)[nodemap]
    rec_all = np.ascontiguousarray(
        rec_all.reshape(N_CORES, BLOCKS, 128).transpose(0, 2, 1)
    )

    wt = np.ascontiguousarray(W.T).astype(BF16)
    bias = np.ascontiguousarray(b.reshape(128, 1))
    ident = np.eye(128, dtype=np.float32)
    iotar = np.ascontiguousarray(
        np.broadcast_to(np.arange(GW, dtype=np.float32), (128, K, GW)).reshape(128, K * GW)
    ).astype(BF16)

    in_maps = [
        {
            "xe": np.ascontiguousarray(X[c]),
            "lid": np.ascontiguousarray(lid_all[c]),
            "recv": rec_all[c],
            "wt": wt,
            "bias": bias,
            "ident": ident,
            "iotar": iotar,
        }
        for c in range(N_CORES)
    ]
    return nodemap, in_maps


def run(edge_data, dst, W, b, trace=False, tmpdir=None):
    from concourse.bass_utils import run_bass_kernel_spmd

    nodemap, in_maps = prepare_inputs(edge_data, dst, W, b)
    nc = _get_module()
    res = run_bass_kernel_spmd(
        nc, in_maps, core_ids=list(range(N_CORES)), trace=trace, tmpdir=tmpdir,
    )
    slots = np.concatenate(
        [res.results[c]["out"].T for c in range(N_CORES)], axis=0
    ).astype(np.float32)                                   # [NPAD, 128] in slot order
    full = np.empty((NPAD, F), np.float32)
    full[nodemap] = slots
    return np.ascontiguousarray(full[:N_NODES]), res


def kernel(edge_data, dst, W, b):
    out, _ = run(edge_data, dst, W, b, trace=False)
    return out
